# revision 8
# baseline (speedup 1.0000x reference)
"""Trainium2 Bass kernel for 3x3 VALID conv: x[32,128,64,64] * w[256,128,3,3] + bias.

Strategy:
  - Data-parallel over batch: 8 cores x 4 images each; weights/bias replicated.
  - Per core: implicit GEMM. Contraction dim = C_IN = 128 = partition dim.
    For each filter tap (u,v), accumulate
        psum[o, p] += W[c, o; u,v].T @ x[c, p + u*64 + v]
    over the flattened output grid of 62 rows x 64 cols (the last 2 cols of
    each row are invalid and trimmed on the host).
  - float32r matmuls (1 cycle/row for free-dim >= 256 vs 4 cycles/row fp32).
  - PSUM -> SBUF evacuation + bias add fused on ScalarE (activation Identity).
"""

import numpy as np

import concourse.bass as bass
import concourse.bacc as bacc
import concourse.tile as tile
from concourse import mybir
from concourse.bass_utils import run_bass_kernel_spmd

N_CORES = 8
B_FULL, C_IN, H, W = 32, 128, 64, 64
C_OUT, KH, KW = 256, 3, 3
B_LOC = B_FULL // N_CORES          # images per core
H_OUT, W_OUT = H - KH + 1, W - KW + 1   # 62, 62
N_HALF = C_OUT // 128              # 2 output-channel halves
ROWS_PER_CHUNK = 8                 # 8 out rows x 64 cols = 512 = one PSUM bank
N_PIX_FLAT = H_OUT * W            # 62*64 = 3968 flattened out positions
X_PAD = (H_OUT + KH - 1) * W + 128  # padded free size so shifted reads stay in-bounds

_cached = {}


def _build_nc():
    f32 = mybir.dt.float32
    f32r = mybir.dt.float32r
    nc = bacc.Bacc()

    x_d = nc.declare_dram_parameter("x", [B_LOC, C_IN, H, W], f32r, isOutput=False)
    w_d = nc.declare_dram_parameter("w", [C_IN, KH * KW, C_OUT], f32r, isOutput=False)
    b_d = nc.declare_dram_parameter("bias_in", [128, N_HALF], f32, isOutput=False)
    y_d = nc.declare_dram_parameter(
        "y", [B_LOC, N_HALF, 128, H_OUT, W], f32, isOutput=True
    )

    with tile.TileContext(nc) as tc:
        with (
            tc.tile_pool(name="const", bufs=1) as cpool,
            tc.tile_pool(name="xin", bufs=2) as xpool,
            tc.tile_pool(name="out", bufs=4) as opool,
            tc.tile_pool(name="psum", bufs=4, space="PSUM") as ppool,
        ):
            w_t = cpool.tile([C_IN, KH * KW, C_OUT], f32r)
            nc.sync.dma_start(w_t[:], w_d[:])
            b_t = cpool.tile([128, N_HALF], f32)
            nc.sync.dma_start(b_t[:], b_d[:])

            n_chunks = (H_OUT + ROWS_PER_CHUNK - 1) // ROWS_PER_CHUNK
            for b in range(B_LOC):
                x_t = xpool.tile([C_IN, X_PAD], f32r)
                nc.sync.dma_start(x_t[:, 0 : H * W], x_d[b])
                # Fill the tail pad with (arbitrary) real data: it only feeds the
                # invalid output columns (j >= 62) that the host trims away.
                x_flat = x_d[b].rearrange("c h w -> c (h w)")
                nc.sync.dma_start(
                    x_t[:, H * W : X_PAD], x_flat[:, 0 : X_PAD - H * W]
                )

                for half in range(N_HALF):
                    for chunk in range(n_chunks):
                        i0 = chunk * ROWS_PER_CHUNK
                        r = min(ROWS_PER_CHUNK, H_OUT - i0)
                        n = r * W
                        p0 = i0 * W
                        ps = ppool.tile([128, ROWS_PER_CHUNK, W], f32, tag="ps")
                        for uv in range(KH * KW):
                            u, v = divmod(uv, KW)
                            shift = p0 + u * W + v
                            nc.tensor.matmul(
                                ps[:, 0:r, :],
                                w_t[:, uv, half * 128 : (half + 1) * 128],
                                x_t[:, shift : shift + n],
                                start=(uv == 0),
                                stop=(uv == KH * KW - 1),
                            )
                        o_t = opool.tile([128, ROWS_PER_CHUNK, W], f32, tag="o")
                        nc.scalar.activation(
                            o_t[:, 0:r, :],
                            ps[:, 0:r, :],
                            mybir.ActivationFunctionType.Identity,
                            bias=b_t[:, half : half + 1],
                        )
                        nc.sync.dma_start(
                            y_d[b, half, :, i0 : i0 + r, :], o_t[:, 0:r, :]
                        )

    nc.compile()
    if not nc.is_finalized():
        nc.finalize()
    return nc


def kernel(inputs, weights, bias, profile=False, trace_kwargs=None):
    inputs = np.ascontiguousarray(inputs, dtype=np.float32)
    # [O, C, KH, KW] -> [C, KH*KW, O]  (lhsT layout: contraction dim on partitions)
    w_t = np.ascontiguousarray(weights.astype(np.float32).transpose(1, 2, 3, 0)).reshape(
        C_IN, KH * KW, C_OUT
    )
    # [C_OUT, 1] -> [128, N_HALF] with bias_sb[p, h] = bias[h*128 + p]
    b_t = np.ascontiguousarray(
        bias.astype(np.float32).reshape(N_HALF, 128).T
    )

    if "nc" not in _cached:
        _cached["nc"] = _build_nc()
    nc = _cached["nc"]

    in_maps = [
        {
            "x": inputs[i * B_LOC : (i + 1) * B_LOC],
            "w": w_t,
            "bias_in": b_t,
        }
        for i in range(N_CORES)
    ]
    res = run_bass_kernel_spmd(
        nc,
        in_maps,
        list(range(N_CORES)),
        trace=profile,
        **(trace_kwargs or {}),
    )
    _cached["last_result"] = res

    shards = []
    for i in range(N_CORES):
        y = res.results[i]["y"]  # [B_LOC, 2, 128, 62, 64]
        shards.append(
            y.reshape(B_LOC, C_OUT, H_OUT, W)[..., :W_OUT]
        )
    return np.ascontiguousarray(np.concatenate(shards, axis=0), dtype=np.float32)


# revision 9
# speedup vs baseline: 1.0423x; 1.0423x over previous
"""Trainium2 Bass kernel for 3x3 VALID conv: x[32,128,64,64] * w[256,128,3,3] + bias.

Strategy:
  - Data-parallel over batch: 8 cores x 4 images each; weights/bias replicated.
  - Per core: implicit GEMM. Contraction dim = C_IN = 128 = partition dim.
    For each filter tap (u,v), accumulate
        psum[o, p] += W[c, o; u,v].T @ x[c, p + u*64 + v]
    over the flattened output grid of 62 rows x 64 cols (the last 2 cols of
    each row are invalid and trimmed on the host).
  - float32r matmuls (1 cycle/row for free-dim >= 256 vs 4 cycles/row fp32).
  - Inputs loaded in pieces so the first matmul group starts after ~1 MB of
    DMA instead of the full image + weights.
  - PSUM -> SBUF evacuation + bias add on VectorE (tensor_scalar_add).
"""

import numpy as np

import concourse.bacc as bacc
import concourse.tile as tile
from concourse import mybir
from concourse.bass_utils import run_bass_kernel_spmd

N_CORES = 8
B_FULL, C_IN, H, W = 32, 128, 64, 64
C_OUT, KH, KW = 256, 3, 3
B_LOC = B_FULL // N_CORES          # images per core
H_OUT, W_OUT = H - KH + 1, W - KW + 1   # 62, 62
N_HALF = C_OUT // 128              # 2 output-channel halves
ROWS_PER_CHUNK = 8                 # 8 out rows x 64 cols = 512 = one PSUM bank
X_PAD = (H_OUT + KH - 1) * W + 128  # padded free size so shifted reads stay in-bounds
X_PIECES = [(0, 1056), (1056, 2112), (2112, 3168), (3168, H * W)]

_cached = {}


def _build_nc():
    f32 = mybir.dt.float32
    f32r = mybir.dt.float32r
    nc = bacc.Bacc()

    x_d = nc.declare_dram_parameter("x", [B_LOC, C_IN, H, W], f32r, isOutput=False)
    w_d = nc.declare_dram_parameter("w", [C_IN, KH * KW, C_OUT], f32r, isOutput=False)
    b_d = nc.declare_dram_parameter("bias_in", [128, N_HALF], f32, isOutput=False)
    y_d = nc.declare_dram_parameter(
        "y", [B_LOC, N_HALF, 128, H_OUT, W], f32, isOutput=True
    )

    with tile.TileContext(nc) as tc:
        with (
            tc.tile_pool(name="const", bufs=1) as cpool,
            tc.tile_pool(name="xin", bufs=2) as xpool,
            tc.tile_pool(name="out", bufs=4) as opool,
            tc.tile_pool(name="psum", bufs=4, space="PSUM") as ppool,
        ):
            w_t = cpool.tile([C_IN, KH * KW, C_OUT], f32r)
            b_t = cpool.tile([128, N_HALF], f32)

            def load_x(b, first):
                x_t = xpool.tile([C_IN, X_PAD], f32r, tag="x")
                x_flat = x_d[b].rearrange("c h w -> c (h w)")
                for k, (lo, hi) in enumerate(X_PIECES):
                    nc.sync.dma_start(x_t[:, lo:hi], x_flat[:, lo:hi])
                    if first and k == 0:
                        # rest of the preamble loads, off the critical path
                        nc.sync.dma_start(
                            w_t[:, :, 128:256], w_d[:, :, 128:256]
                        )
                        nc.sync.dma_start(b_t[:], b_d[:])
                # Tail pad: (arbitrary) real data — feeds only the invalid
                # output columns (j >= 62) that the host trims away.
                nc.sync.dma_start(
                    x_t[:, H * W : X_PAD], x_flat[:, 0 : X_PAD - H * W]
                )
                return x_t

            # critical path for the first matmul group: w half0 + x piece0
            nc.sync.dma_start(w_t[:, :, 0:128], w_d[:, :, 0:128])

            n_chunks = (H_OUT + ROWS_PER_CHUNK - 1) // ROWS_PER_CHUNK
            for b in range(B_LOC):
                x_t = load_x(b, first=(b == 0))
                for chunk in range(n_chunks):
                    i0 = chunk * ROWS_PER_CHUNK
                    r = min(ROWS_PER_CHUNK, H_OUT - i0)
                    n = r * W
                    p0 = i0 * W
                    for half in range(N_HALF):
                        ps = ppool.tile([128, ROWS_PER_CHUNK, W], f32, tag="ps")
                        for uv in range(KH * KW):
                            u, v = divmod(uv, KW)
                            shift = p0 + u * W + v
                            nc.tensor.matmul(
                                ps[:, 0:r, :],
                                w_t[:, uv, half * 128 : (half + 1) * 128],
                                x_t[:, shift : shift + n],
                                start=(uv == 0),
                                stop=(uv == KH * KW - 1),
                            )
                        o_t = opool.tile([128, ROWS_PER_CHUNK, W], f32, tag="o")
                        nc.vector.tensor_scalar_add(
                            o_t[:, 0:r, :], ps[:, 0:r, :], b_t[:, half : half + 1]
                        )
                        nc.sync.dma_start(
                            y_d[b, half, :, i0 : i0 + r, :], o_t[:, 0:r, :]
                        )

    nc.compile()
    if not nc.is_finalized():
        nc.finalize()
    return nc


def kernel(inputs, weights, bias, profile=False, trace_kwargs=None):
    inputs = np.ascontiguousarray(inputs, dtype=np.float32)
    # [O, C, KH, KW] -> [C, KH*KW, O]  (lhsT layout: contraction dim on partitions)
    w_t = np.ascontiguousarray(weights.astype(np.float32).transpose(1, 2, 3, 0)).reshape(
        C_IN, KH * KW, C_OUT
    )
    # [C_OUT, 1] -> [128, N_HALF] with bias_sb[p, h] = bias[h*128 + p]
    b_t = np.ascontiguousarray(
        bias.astype(np.float32).reshape(N_HALF, 128).T
    )

    if "nc" not in _cached:
        _cached["nc"] = _build_nc()
    nc = _cached["nc"]

    in_maps = [
        {
            "x": inputs[i * B_LOC : (i + 1) * B_LOC],
            "w": w_t,
            "bias_in": b_t,
        }
        for i in range(N_CORES)
    ]
    res = run_bass_kernel_spmd(
        nc,
        in_maps,
        list(range(N_CORES)),
        trace=profile,
        **(trace_kwargs or {}),
    )
    _cached["last_result"] = res

    shards = []
    for i in range(N_CORES):
        y = res.results[i]["y"]  # [B_LOC, 2, 128, 62, 64]
        shards.append(
            y.reshape(B_LOC, C_OUT, H_OUT, W)[..., :W_OUT]
        )
    return np.ascontiguousarray(np.concatenate(shards, axis=0), dtype=np.float32)
